# revision 1
# baseline (speedup 1.0000x reference)
"""Trainium2 Bass kernel for the supervised-contrastive loss (nn_KCL_69784628626020).

Strategy (8 NeuronCores, SPMD):
  - Shard anchors (rows of q, k, y) across cores: 1024 rows/core.
  - Each core computes its [1024, 8192] slab of the score matrix
    S = q_loc @ q_full^T on the tensor engine (float32r, full rate at N>=512).
  - The per-column weight w_j = 1/count(y_j) is folded into the matmul as an
    extra K=1 rank-1 update adding TAU*ln(w_j) to the scores, so that the
    scalar engine's exp(PSUM/TAU) directly produces EW_ij = exp(S_ij/TAU)*w_j.
  - Per row i:
        A_i = sum_j  EW_ij            (diag excluded)
        B_i = sum_{y_j==y_i} EW_ij    (diag excluded)
        den_i = log(A_i - B_i)
        num_i = log(kpos_i + c_i * B_i)      # c_i = count(y_i), B*c = unweighted
        loss_i = (den_i - num_i) / (c_i - 1 + K)
    A and B each come from ONE fused DVE scalar_tensor_tensor op per tile
    (compare + multiply + row-reduce).  Diagonal exclusion is data-driven
    (host-provided global row ids compared against a column iota), so the
    program is identical across cores (SPMD-safe).
  - Class counts are computed on device: row-sums of the y-equality mask give
    count(y_i) for local rows; an AllGather assembles counts for all 8192
    columns.
  - kpos_i = sum_k exp(q_i . k_ik / TAU) via fused multiply-reduce per k.
  - Final mean: per-core partial sum via a ones-matmul partition reduction;
    host adds the 8 partials (the unshard step).
"""

import numpy as np
from contextlib import ExitStack

import concourse.bass as bass
import concourse.bacc as bacc
import concourse.tile as tile
from concourse import mybir
from concourse.bass_utils import run_bass_kernel_spmd
import ml_dtypes

F32 = mybir.dt.float32
F32R = mybir.dt.float32r
F16 = mybir.dt.float16
BF16 = mybir.dt.bfloat16

TAU = 0.07
NCORES = 8


class Cfg:
    def __init__(self, N=8192, D=512, KP=8, TW=1024):
        self.N = N            # total rows (anchors)
        self.D = D            # feature dim
        self.KP = KP          # external positives per anchor
        self.TW = TW          # column tile width
        self.NL = N // NCORES     # rows per core
        self.NB = self.NL // 128  # row blocks per core
        self.NS = N // TW         # column tiles
        self.KC = D // 128        # contraction chunks
        assert self.NL % 128 == 0 and N % TW == 0 and D % 128 == 0
        assert TW % 512 == 0
        self.NCH = TW // 512      # 512-wide matmul chunks per column tile


# Engine selection knobs (tuned from traces).
STT1_ENGINES = None  # set in build_bass
STT2_ENGINES = None
KPATH_ENGINES = None


def build_bass(cfg: Cfg, stt1_eng="vector", stt2_eng="vector", k_eng="vector"):
    N, D, KP, TW = cfg.N, cfg.D, cfg.KP, cfg.TW
    NL, NB, NS, KC, NCH = cfg.NL, cfg.NB, cfg.NS, cfg.KC, cfg.NCH

    nc = bacc.Bacc("TRN2", target_bir_lowering=False, debug=False,
                   num_devices=NCORES)

    # ---- kernel I/O -------------------------------------------------------
    qT_d = nc.dram_tensor("qT", [KC, 128, N], F32R, kind="ExternalInput")
    qTl_d = nc.dram_tensor("qTl", [KC, 128, NL], F32R, kind="ExternalInput")
    kr_d = nc.dram_tensor("kr", [NB, 128, KP * D], BF16, kind="ExternalInput")
    qr_d = nc.dram_tensor("qr", [NB, 128, D], F32, kind="ExternalInput")
    ybc_d = nc.dram_tensor("ybc", [128, N], F16, kind="ExternalInput")
    yrow_d = nc.dram_tensor("yrow", [128, NB], F32, kind="ExternalInput")
    colid_d = nc.dram_tensor("colid", [128, TW], F16, kind="ExternalInput")
    rowadj_d = nc.dram_tensor("rowadj", [128, NB * NS], F32, kind="ExternalInput")
    out_d = nc.dram_tensor("out", [1, 1], F32, kind="ExternalOutput")

    eng = {"vector": nc.vector, "gpsimd": nc.gpsimd}
    stt1e = eng[stt1_eng]
    stt2e = eng[stt2_eng]
    ke = eng[k_eng]

    with tile.TileContext(nc) as tc, ExitStack() as ctx:
        const = ctx.enter_context(tc.tile_pool(name="const", bufs=1))
        rh_pool = ctx.enter_context(tc.tile_pool(name="rh", bufs=8))
        psum_pool = ctx.enter_context(tc.tile_pool(name="ps", bufs=3, space="PSUM"))
        ew_pool = ctx.enter_context(tc.tile_pool(name="ew", bufs=3))
        t1_pool = ctx.enter_context(tc.tile_pool(name="t1", bufs=3))
        t2_pool = ctx.enter_context(tc.tile_pool(name="t2", bufs=2))
        k_pool = ctx.enter_context(tc.tile_pool(name="kp", bufs=2))
        q_pool = ctx.enter_context(tc.tile_pool(name="qp", bufs=2))
        dram = ctx.enter_context(tc.tile_pool(name="dram", bufs=1, space="DRAM"))

        # ---- resident constants ------------------------------------------
        qtl = [const.tile([128, NL], F32R, tag=f"qtl{c}", name=f"qtl{c}") for c in range(KC)]
        for c in range(KC):
            nc.sync.dma_start(qtl[c][:, :], qTl_d[c, :, :])
        ybc = const.tile([128, N], F16, tag="ybc")
        nc.sync.dma_start(ybc[:, :], ybc_d[:, :])
        colid = const.tile([128, TW], F16, tag="colid")
        nc.sync.dma_start(colid[:, :], colid_d[:, :])
        yrow = const.tile([128, NB], F32, tag="yrow")
        nc.sync.dma_start(yrow[:, :], yrow_d[:, :])
        rowadj = const.tile([128, NB * NS], F32, tag="rowadj")
        nc.sync.dma_start(rowadj[:, :], rowadj_d[:, :])

        ones_k1 = const.tile([1, 128], F32R, tag="ones_k1")
        nc.vector.memset(ones_k1[:, :].bitcast(F32), 1.0)
        ones_col = const.tile([128, 1], F32, tag="ones_col")
        nc.vector.memset(ones_col[:, :], 1.0)

        # accumulator slots
        aslt = const.tile([128, NB * NS], F32, tag="aslt")
        bslt = const.tile([128, NB * NS], F32, tag="bslt")
        kss = const.tile([128, NB * KP], F32, tag="kss")
        kpos = const.tile([128, NB], F32, tag="kpos")
        cloc = const.tile([128, NB], F32, tag="cloc")
        losscol = const.tile([128, NB], F32, tag="losscol")

        # ---- phase W: class counts + lw ----------------------------------
        cnt_scr = const.tile([128, N], F16, tag="cnt_scr")
        for b in range(NB):
            nc.vector.tensor_scalar(
                cnt_scr[:, :], ybc[:, :], yrow[:, b:b + 1], None,
                op0=mybir.AluOpType.is_equal,
                op1=mybir.AluOpType.add,
                accum_out=cloc[:, b:b + 1])

        cpart = dram.tile([1, NL], F32)
        call = dram.tile([NCORES, NL], F32, addr_space="Shared")
        # cpart[0, b*128+p] = cloc[p, b]
        nc.sync.dma_start(
            cpart[:, :].rearrange("o (b p) -> p (o b)", b=NB, p=128),
            cloc[:, :])
        nc.gpsimd.collective_compute(
            "AllGather", mybir.AluOpType.bypass,
            ins=[cpart[:, :].opt()],
            outs=[call[:, :].opt()],
            replica_groups=[list(range(NCORES))],
        )
        # counts for all N columns -> SBUF [128, N/128] (global row-major)
        NF = N // 128
        csb = const.tile([128, NF], F32, tag="csb")
        nc.sync.dma_start(
            csb[:, :],
            call[:, :].rearrange("r l -> (r l)").rearrange("(p f) -> p f", p=128, f=NF))
        lnc = const.tile([128, NF], F32, tag="lnc")
        nc.scalar.activation(lnc[:, :], csb[:, :], mybir.ActivationFunctionType.Ln)
        lwsb = const.tile([128, NF], F32R, tag="lwsb")
        nc.vector.tensor_scalar_mul(lwsb[:, :], lnc[:, :], -TAU)
        lw_d = dram.tile([1, N], F32R)
        nc.sync.dma_start(
            lw_d[:, :].rearrange("o (p f) -> p (o f)", p=128, f=NF),
            lwsb[:, :])
        lwrow = const.tile([1, N], F32R, tag="lwrow")
        nc.sync.dma_start(lwrow[:, :], lw_d[:, :])

        # ---- main loop: score slab ---------------------------------------
        for s in range(NS):
            rhs = [rh_pool.tile([128, TW], F32R, tag="rh", name=f"rhs{s}_{c2}") for c2 in range(KC)]
            for c in range(KC):
                nc.sync.dma_start(rhs[c][:, :], qT_d[c, :, s * TW:(s + 1) * TW])
            for b in range(NB):
                ps = psum_pool.tile([128, TW], F32)
                for nch in range(NCH):
                    o = ps[:, nch * 512:(nch + 1) * 512]
                    for c in range(KC):
                        nc.tensor.matmul(
                            o,
                            qtl[c][:, b * 128:(b + 1) * 128],
                            rhs[c][:, nch * 512:(nch + 1) * 512],
                            start=(c == 0), stop=False)
                    nc.tensor.matmul(
                        o,
                        ones_k1[0:1, :],
                        lwrow[0:1, s * TW + nch * 512: s * TW + (nch + 1) * 512],
                        start=False, stop=True)
                ew = ew_pool.tile([128, TW], F32)
                nc.scalar.activation(ew[:, :], ps[:, :],
                                     mybir.ActivationFunctionType.Exp,
                                     scale=float(1.0 / TAU))
                # A: zero the diagonal, row-sum everything
                t1 = t1_pool.tile([128, TW], F32)
                stt1e.scalar_tensor_tensor(
                    t1[:, :], colid[:, :], rowadj[:, (b * NS + s):(b * NS + s) + 1],
                    ew[:, :],
                    op0=mybir.AluOpType.not_equal, op1=mybir.AluOpType.mult,
                    accum_out=aslt[:, (b * NS + s):(b * NS + s) + 1])
                # B: same-class row-sum (diag already zeroed in t1)
                t2 = t2_pool.tile([128, TW], F16)
                stt2e.scalar_tensor_tensor(
                    t2[:, :], ybc[:, s * TW:(s + 1) * TW], yrow[:, b:b + 1],
                    t1[:, :],
                    op0=mybir.AluOpType.is_equal, op1=mybir.AluOpType.mult,
                    accum_out=bslt[:, (b * NS + s):(b * NS + s) + 1])

        # ---- k-path: kpos = sum_k exp(q.k/TAU) ---------------------------
        for b in range(NB):
            kt = k_pool.tile([128, KP * D], BF16, tag="kt")
            nc.sync.dma_start(kt[:, :], kr_d[b, :, :])
            qt = q_pool.tile([128, D], F32, tag="qt")
            nc.sync.dma_start(qt[:, :], qr_d[b, :, :])
            for kk in range(KP):
                kscr = q_pool.tile([128, D], BF16, tag="kscr")
                ke.scalar_tensor_tensor(
                    kscr[:, :], kt[:, kk * D:(kk + 1) * D], 1.0,
                    qt[:, :],
                    op0=mybir.AluOpType.mult, op1=mybir.AluOpType.mult,
                    accum_out=kss[:, b * KP + kk: b * KP + kk + 1])
            ksse = const.tile([128, KP], F32, tag=f"ksse{b}")
            nc.scalar.activation(
                ksse[:, :],
                kss[:, b * KP:(b + 1) * KP],
                mybir.ActivationFunctionType.Exp, scale=float(1.0 / TAU),
                accum_out=kpos[:, b:b + 1])

        # ---- finalize per row block --------------------------------------
        fin = const.tile([128, 6 * NB], F32, tag="fin")
        for b in range(NB):
            acol = fin[:, 6 * b + 0: 6 * b + 1]
            bcol = fin[:, 6 * b + 1: 6 * b + 2]
            nc.vector.tensor_reduce(acol, aslt[:, b * NS:(b + 1) * NS],
                                    mybir.AxisListType.X, mybir.AluOpType.add)
            nc.vector.tensor_reduce(bcol, bslt[:, b * NS:(b + 1) * NS],
                                    mybir.AxisListType.X, mybir.AluOpType.add)
            den_in = fin[:, 6 * b + 2: 6 * b + 3]
            nc.vector.tensor_sub(den_in, acol, bcol)
            num_in = fin[:, 6 * b + 3: 6 * b + 4]
            # num_in = kpos + cloc * B
            nc.vector.scalar_tensor_tensor(
                num_in, bcol, cloc[:, b:b + 1], kpos[:, b:b + 1],
                op0=mybir.AluOpType.mult, op1=mybir.AluOpType.add)
            den_l = fin[:, 6 * b + 4: 6 * b + 5]
            nc.scalar.activation(den_l, den_in, mybir.ActivationFunctionType.Ln)
            num_l = fin[:, 6 * b + 5: 6 * b + 6]
            nc.scalar.activation(num_l, num_in, mybir.ActivationFunctionType.Ln)
        # losscol[:, b] = (den_l - num_l) / (cloc - 1 + KP)
        dinv_t = const.tile([128, NB], F32, tag="dinv")
        tmp_t = const.tile([128, NB], F32, tag="tmpd")
        nc.vector.tensor_scalar_add(tmp_t[:, :], cloc[:, :], float(KP - 1))
        nc.vector.reciprocal(dinv_t[:, :], tmp_t[:, :])
        for b in range(NB):
            den_l = fin[:, 6 * b + 4: 6 * b + 5]
            num_l = fin[:, 6 * b + 5: 6 * b + 6]
            diff = fin[:, 6 * b + 2: 6 * b + 3]  # overwrite den_in
            nc.vector.tensor_sub(diff, den_l, num_l)
            nc.vector.tensor_mul(losscol[:, b:b + 1], diff, dinv_t[:, b:b + 1])

        # ---- reduce to a single partial ----------------------------------
        lsum = const.tile([128, 1], F32, tag="lsum")
        nc.vector.tensor_reduce(lsum[:, :], losscol[:, :],
                                mybir.AxisListType.X, mybir.AluOpType.add)
        psf = psum_pool.tile([128, 512], F32, bufs=1)
        nc.tensor.matmul(psf[0:1, 0:1], lsum[:, :],
                         ones_col[:, :], start=True, stop=True)
        outsb = const.tile([1, 1], F32, tag="outsb")
        nc.scalar.copy(outsb[0:1, 0:1], psf[0:1, 0:1])
        nc.sync.dma_start(out_d[:, :], outsb[0:1, 0:1])

    nc.compile()
    return nc


# ---------------------------------------------------------------------------
# host-side marshalling
# ---------------------------------------------------------------------------

def make_inputs(q, k, y, cfg: Cfg):
    """Build the per-core input maps (pure layout/replication marshalling)."""
    N, D, KP, TW = cfg.N, cfg.D, cfg.KP, cfg.TW
    NL, NB, NS, KC = cfg.NL, cfg.NB, cfg.NS, cfg.KC
    q = np.asarray(q, dtype=np.float32)
    k = np.asarray(k, dtype=np.float32)
    y = np.asarray(y)

    qT = np.ascontiguousarray(q.T).reshape(KC, 128, N)
    ybc = np.broadcast_to(y.astype(np.float16)[None, :], (128, N)).copy()
    colid = np.broadcast_to(np.arange(TW, dtype=np.float16)[None, :], (128, TW)).copy()

    in_maps = []
    for r in range(NCORES):
        rows = slice(r * NL, (r + 1) * NL)
        qTl = np.ascontiguousarray(q[rows].T).reshape(KC, 128, NL)
        kr = np.ascontiguousarray(k[rows].reshape(NB, 128, KP * D)).astype(ml_dtypes.bfloat16)
        qr = np.ascontiguousarray(q[rows].reshape(NB, 128, D))
        yrow = np.ascontiguousarray(y[rows].astype(np.float32).reshape(NB, 128).T)
        # rowadj[p, b*NS+s] = global_row - s*TW
        p = np.arange(128, dtype=np.float32)
        badx = np.arange(NB, dtype=np.float32)
        sadx = np.arange(NS, dtype=np.float32)
        grow = r * NL + badx[:, None, None] * 128 + p[None, :, None]  # [NB,128,1]
        rowadj = (grow - sadx[None, None, :] * TW)                   # [NB,128,NS]
        rowadj = np.ascontiguousarray(rowadj.transpose(1, 0, 2).reshape(128, NB * NS),
                                      dtype=np.float32)
        in_maps.append({
            "qT": qT, "qTl": qTl, "kr": kr, "qr": qr,
            "ybc": ybc, "yrow": yrow, "colid": colid, "rowadj": rowadj,
        })
    return in_maps


_CACHE = {}


def _get_nc(cfg_key):
    if cfg_key not in _CACHE:
        cfg = Cfg()
        _CACHE[cfg_key] = (cfg, build_bass(cfg))
    return _CACHE[cfg_key]


def kernel(q, k, y, trace=False):
    cfg, nc = _get_nc("full")
    in_maps = make_inputs(q, k, y, cfg)
    res = run_bass_kernel_spmd(nc, in_maps, core_ids=list(range(NCORES)),
                               trace=trace)
    total = np.sum([res.results[r]["out"][0, 0] for r in range(NCORES)],
                   dtype=np.float64)
    out = np.asarray(total / cfg.N, dtype=np.float32)
    if trace:
        kernel.last_results = res
    return out



# revision 23
# speedup vs baseline: 1.6587x; 1.6587x over previous
"""Trainium2 Bass kernel for the supervised-contrastive loss (nn_KCL_69784628626020).

Strategy (8 NeuronCores, SPMD), v2:
  - Shard anchors (rows of q, k, y) across cores: 1024 rows/core.
  - Each core computes its [1024, 8192] slab of E = exp(q_loc @ q_full^T / TAU)
    on the tensor engine (bf16 operands, fp32 PSUM), in 4 column-"quarters"
    of 2048 per 128-row block.  The full q^T is SBUF-resident; the stationary
    (lhsT) operand is a slice of the same resident tensor, so DMA is ~8MB.
  - COLUMN ROTATION: core r's column order is rolled by r*1024 so that the
    self-similarity (diagonal) entry of local row-block b always lands in the
    static window [b*128, (b+1)*128).  A tiny eye-masked reduce extracts the
    exact stored E_ii per row; no per-tile diagonal masking is needed.
  - Per row i (sums include the diagonal; it cancels exactly):
        SC_i = sum_{y_j==y_i} E_ij        (DVE fused masked reduce / quarter)
        W_i  = sum_j w_j E_ij             (GpSimd product tile + ACT accum;
                                           w_j = 1/count(y_j) in f16)
        den_i = log(W_i - w_i*SC_i)       (diagonal + same-class terms cancel)
        num_i = log(kpos_i + SC_i - E_ii)
        loss_i = (den_i - num_i) / (count_i - 1 + K)
  - Class counts are computed on device (DVE label-equality reduces), shared
    via a tiny f16 AllGather of per-row 1/count, and re-broadcast into each
    core's rotated column order with one indirect DMA whose chunk indices are
    host-provided data (SPMD-safe).
  - kpos_i = sum_k exp(q_i . k_ik / TAU): GpSimd multiplies, ACT accumulates
    + exponentiates.
  - Final mean: ones-matmul partition reduction; host adds the 8 partials.
"""

import numpy as np
from contextlib import ExitStack

import concourse.bass as bass
import concourse.bacc as bacc
import concourse.tile as tile
from concourse import mybir
from concourse.bass import IndirectOffsetOnAxis
from concourse.bass_utils import run_bass_kernel_spmd
import ml_dtypes

F32 = mybir.dt.float32
F16 = mybir.dt.float16
BF16 = mybir.dt.bfloat16
I32 = mybir.dt.int32
AL = mybir.AluOpType
AF = mybir.ActivationFunctionType

TAU = 0.07
NCORES = 8


class Cfg:
    def __init__(self, N=8192, D=512, KP=8, NQ=4):
        self.N, self.D, self.KP, self.NQ = N, D, KP, NQ
        self.NL = N // NCORES      # rows per core
        self.NB = self.NL // 128   # 128-row blocks per core
        self.KC = D // 128         # contraction chunks
        self.QW = N // NQ          # column quarter width
        self.NCH = max(1, self.QW // 512)
        self.CW = self.QW // self.NCH   # matmul chunk width
        assert self.NL % 128 == 0 and self.QW % self.NCH == 0
        assert self.CW <= 512


def build_bass(cfg: Cfg, debug_out=False):
    N, D, KP, NQ = cfg.N, cfg.D, cfg.KP, cfg.NQ
    NL, NB, KC, QW, NCH, CW = cfg.NL, cfg.NB, cfg.KC, cfg.QW, cfg.NCH, cfg.CW
    NQT = NB * NQ              # total quarters
    WLAG = min(NQT - 1, 12)    # ACT-queue lag before W reductions start

    nc = bacc.Bacc("TRN2", target_bir_lowering=False, debug=False,
                   num_devices=NCORES)

    # ---- kernel I/O -------------------------------------------------------
    qT_d = nc.dram_tensor("qT", [KC, 128, N], BF16, kind="ExternalInput")
    kr_d = nc.dram_tensor("kr", [NB, 128, KP * D], BF16, kind="ExternalInput")
    qr_d = nc.dram_tensor("qr", [NB, 128, D], BF16, kind="ExternalInput")
    ybc_d = nc.dram_tensor("ybc", [128, N], F16, kind="ExternalInput")
    yrow_d = nc.dram_tensor("yrow", [128, NB], F32, kind="ExternalInput")
    eyem_d = nc.dram_tensor("eyem", [128, 128], F16, kind="ExternalInput")
    mask8_d = nc.dram_tensor("mask8", [128, 1], F32, kind="ExternalInput")
    out_d = nc.dram_tensor("out", [1, 1], F32, kind="ExternalOutput")
    if debug_out:
        dwbc_d = nc.dram_tensor("dwbc", [128, N], F16, kind="ExternalOutput")
        dfin_d = nc.dram_tensor("dfin", [128, 8 * NB], F32, kind="ExternalOutput")

    with tile.TileContext(nc) as tc, ExitStack() as ctx:
        const = ctx.enter_context(tc.tile_pool(name="const", bufs=1))
        ew_pool = ctx.enter_context(tc.tile_pool(name="ew", bufs=3))
        psum_pool = ctx.enter_context(tc.tile_pool(name="ps", bufs=2, space="PSUM"))
        k_pool = ctx.enter_context(tc.tile_pool(name="kp", bufs=1))
        q_pool = ctx.enter_context(tc.tile_pool(name="qp", bufs=2))
        ks_pool = ctx.enter_context(tc.tile_pool(name="ks", bufs=1))
        ws_pool = ctx.enter_context(tc.tile_pool(name="ws", bufs=2))
        dram = ctx.enter_context(tc.tile_pool(name="dram", bufs=1, space="DRAM"))

        # ---- resident constants ------------------------------------------
        # column-quarter-major loads so the PE can start after ~1/4 of qT
        qts = [const.tile([128, N], BF16, tag=f"qts{c}", name=f"qts{c}")
               for c in range(KC)]
        ybc = const.tile([128, N], F16, tag="ybc")
        nc.sync.dma_start(ybc[:, 0:QW], ybc_d[:, 0:QW])
        for c in range(KC):
            nc.sync.dma_start(qts[c][:, 0:QW], qT_d[c, :, 0:QW])
        nc.sync.dma_start(ybc[:, QW:N], ybc_d[:, QW:N])
        # k-path inputs early so GpSimd products (and ACT reduces) start soon
        kts = []
        qts_k = []
        for b in range(NB):
            kt = k_pool.tile([128, KP * D], BF16, tag="kt", name=f"kt{b}")
            nc.sync.dma_start(kt[:, :], kr_d[b, :, :])
            qt = q_pool.tile([128, D], BF16, tag="qt", name=f"qt{b}")
            nc.sync.dma_start(qt[:, :], qr_d[b, :, :])
            kts.append(kt)
            qts_k.append(qt)
        for qq in range(1, NQ):
            for c in range(KC):
                nc.sync.dma_start(qts[c][:, qq * QW:(qq + 1) * QW],
                                  qT_d[c, :, qq * QW:(qq + 1) * QW])
        yrow = const.tile([128, NB], F32, tag="yrow")
        nc.sync.dma_start(yrow[:, :], yrow_d[:, :])
        eyem = const.tile([128, 128], F16, tag="eyem")
        nc.sync.dma_start(eyem[:, :], eyem_d[:, :])
        mask8 = const.tile([128, 1], F32, tag="mask8")
        nc.sync.dma_start(mask8[:, :], mask8_d[:, :])

        ones_col = const.tile([128, 1], F32, tag="ones_col")
        nc.vector.memset(ones_col[:, :], 1.0)

        # accumulator slots
        scs = const.tile([128, NQT], F32, tag="scs")
        ws = const.tile([128, NQT], F32, tag="ws")
        eh = const.tile([128, NB], F32, tag="eh")
        kss = const.tile([128, NB * KP], F32, tag="kss")
        kpos = const.tile([128, NB], F32, tag="kpos")
        cloc = const.tile([128, NB], F32, tag="cloc")
        clocB = const.tile([128, NB], F32, tag="clocB")

        # ---- k-path products on GpSimd (kt, qt ready early) --------------
        kscrs = []
        for b in range(NB):
            kscr = ks_pool.tile([128, KP * D], BF16, tag="kscr", name=f"kscr{b}")
            for kk in range(KP):
                nc.gpsimd.tensor_tensor(
                    kscr[:, kk * D:(kk + 1) * D],
                    kts[b][:, kk * D:(kk + 1) * D], qts_k[b][:, :], op=AL.mult)
            kscrs.append(kscr)

        # ---- class counts on DVE (two halves per block) ------------------
        cnt_scr = const.tile([128, N // 2], F16, tag="cnt_scr")
        H = N // 2
        for b in range(NB):
            nc.vector.tensor_scalar(
                cnt_scr[:, :], ybc[:, 0:H], yrow[:, b:b + 1], None,
                op0=AL.is_equal, op1=AL.add, accum_out=cloc[:, b:b + 1])
        for b in range(NB):
            nc.vector.tensor_scalar(
                cnt_scr[:, :], ybc[:, H:N], yrow[:, b:b + 1], None,
                op0=AL.is_equal, op1=AL.add, accum_out=clocB[:, b:b + 1])
        nc.vector.tensor_add(cloc[:, :], cloc[:, :], clocB[:, :])

        # ---- w vector: wloc = 1/cloc; share via AllGather ----------------
        wloc = const.tile([128, NB], F32, tag="wloc")
        wloc16 = const.tile([128, NB], F16, tag="wloc16")
        wlup = const.tile([128, NB], F32, tag="wlup")
        nc.vector.reciprocal(wloc[:, :], cloc[:, :])
        nc.vector.tensor_copy(wloc16[:, :], wloc[:, :])
        # re-upcast of the f16 w for exact den cancellation
        nc.vector.tensor_copy(wlup[:, :], wloc16[:, :])

        # transpose wloc16 [128, NB] -> 32x32-block layout for a clean DMA
        wpad = const.tile([128, 32], F16, tag="wpad")
        nc.vector.memset(wpad[:, :], 0.0)
        nc.vector.tensor_copy(wpad[:, 0:NB], wloc16[:, :])
        wT = const.tile([128, 32], F16, tag="wT")
        nc.vector.transpose(wT[:, :], wpad[:, :])
        # wpart[0, i*128 + a*32 + j] = w(local row i*128+a*32+j) = wT[a*32+i, j]
        wpart = dram.tile([1, NL], F16)
        dst3 = wpart[:, :].rearrange("o (i a j) -> a i (o j)", i=NB, a=4, j=32)
        for a in range(4):
            nc.gpsimd.dma_start(dst3[a], wT[a * 32:a * 32 + NB, 0:32])
        # rotated share via ReduceScatter: core r contributes w_r to shard c
        # slot (r-c)%8; the scatter hands core c exactly its rotated vector.
        bufc = const.tile([128, NL], F16, tag="bufc")
        nc.gpsimd.dma_start(bufc[0:64, :],
                            wpart[0:1, :].partition_broadcast(64))
        nc.vector.tensor_scalar_mul(bufc[0:64, :], bufc[0:64, :],
                                    mask8[0:64, 0:1])
        bufc_d = dram.tile([1, 64 * NL], F16)
        nc.gpsimd.dma_start(
            bufc_d[:, :].rearrange("o (p g) -> p (o g)", p=64, g=NL),
            bufc[0:64, :])
        wrs_d = dram.tile([1, NCORES * NL], F16)
        nc.gpsimd.collective_compute(
            "ReduceScatter", AL.add,
            ins=[bufc_d[:, :].opt()],
            outs=[wrs_d[:, :].opt()],
            replica_groups=[list(range(NCORES))],
        )
        wbc = const.tile([128, N], F16, tag="wbc")
        nc.gpsimd.dma_start(wbc[:, :], wrs_d[0:1, :].partition_broadcast(128))

        # ---- main loop ----------------------------------------------------
        # PE: quarters of the score slab.  ACT: exp (+ k-path reduces early,
        # W reduces lagged).  DVE: SC masked reduce + diag extract.
        # GpSimd: W product tiles.
        sc_scr = const.tile([128, QW], BF16, tag="sc_scr")
        eh_scr = const.tile([128, 128], BF16, tag="eh_scr")
        kdump = const.tile([128, D], BF16, tag="kdump")
        wdump = const.tile([128, QW], F32, tag="wdump")
        ew_tiles = {}
        wscr_tiles = {}

        kred_jobs = [(b, kk) for b in range(NB) for kk in range(KP)]
        kred_pos = 0
        kred_per_q = (len(kred_jobs) + WLAG - 2) // max(1, WLAG - 1)

        def emit_kred(n):
            nonlocal kred_pos
            for _ in range(n):
                if kred_pos >= len(kred_jobs):
                    return
                b, kk = kred_jobs[kred_pos]
                nc.scalar.activation(
                    kdump[:, :],
                    kscrs[b][:, kk * D:(kk + 1) * D],
                    AF.Copy, accum_out=kss[:, b * KP + kk: b * KP + kk + 1])
                kred_pos += 1

        def emit_wred(m):
            wscr = wscr_tiles.pop(m)
            nc.scalar.activation(
                wdump[:, :], wscr[:, :], AF.Copy,
                accum_out=ws[:, m:m + 1])

        for b in range(NB):
            ew = ew_pool.tile([128, N], BF16, tag="ew", name=f"ew{b}")
            ew_tiles[b] = ew
            for qq in range(NQ):
                m = b * NQ + qq
                ps = psum_pool.tile([128, QW], F32, tag="ps")
                for c in range(KC):
                    for ch in range(NCH):
                        o = ps[:, ch * CW:(ch + 1) * CW]
                        nc.tensor.matmul(
                            o,
                            qts[c][:, b * 128:(b + 1) * 128],
                            qts[c][:, qq * QW + ch * CW: qq * QW + (ch + 1) * CW],
                            start=(c == 0), stop=(c == KC - 1))
                ewq = ew[:, qq * QW:(qq + 1) * QW]
                nc.scalar.activation(ewq, ps[:, :], AF.Exp,
                                     scale=float(1.0 / TAU))
                # ACT queue fillers: k-path reduces early, W reduces lagged
                if m >= 1:
                    emit_kred(kred_per_q)
                if m >= WLAG:
                    emit_wred(m - WLAG)
                # SC: same-class row-sum (incl diag) on DVE
                nc.vector.scalar_tensor_tensor(
                    sc_scr[:, :], ybc[:, qq * QW:(qq + 1) * QW],
                    yrow[:, b:b + 1], ewq,
                    op0=AL.is_equal, op1=AL.mult,
                    accum_out=scs[:, m:m + 1])
                # W product tile on GpSimd
                wscr = ws_pool.tile([128, QW], F32, tag="wscr", name=f"wscr{m}")
                nc.gpsimd.tensor_tensor(
                    wscr[:, :], ewq, wbc[:, qq * QW:(qq + 1) * QW], op=AL.mult)
                wscr_tiles[m] = wscr
            # exact diagonal extraction from the static rotated window
            nc.vector.scalar_tensor_tensor(
                eh_scr[:, :], eyem[:, :], 1.0, ew[:, b * 128:(b + 1) * 128],
                op0=AL.mult, op1=AL.mult,
                accum_out=eh[:, b:b + 1])

        emit_kred(len(kred_jobs))
        for m in range(max(0, NQT - WLAG), NQT):
            emit_wred(m)

        # ---- k-path exps --------------------------------------------------
        ksse = const.tile([128, NB * KP], F32, tag="ksse")
        for b in range(NB):
            nc.scalar.activation(
                ksse[:, b * KP:(b + 1) * KP],
                kss[:, b * KP:(b + 1) * KP],
                AF.Exp, scale=float(1.0 / TAU),
                accum_out=kpos[:, b:b + 1])

        # ---- finalize (wide [128, NB] ops) --------------------------------
        SC = const.tile([128, NB], F32, tag="SC")
        W = const.tile([128, NB], F32, tag="W")
        nc.vector.tensor_reduce(
            SC[:, :], scs[:, :].rearrange("p (b q) -> p b q", b=NB, q=NQ),
            mybir.AxisListType.X, AL.add)
        nc.vector.tensor_reduce(
            W[:, :], ws[:, :].rearrange("p (b q) -> p b q", b=NB, q=NQ),
            mybir.AxisListType.X, AL.add)
        numin = const.tile([128, NB], F32, tag="numin")
        tmp = const.tile([128, NB], F32, tag="tmp")
        densub = const.tile([128, NB], F32, tag="densub")
        # num_in = kpos + (SC - eh)
        nc.vector.tensor_sub(tmp[:, :], SC[:, :], eh[:, :])
        nc.vector.tensor_add(numin[:, :], tmp[:, :], kpos[:, :])
        # den_in = W - w_i * SC  (w_i upcast from the same f16 used in wbc)
        nc.vector.tensor_mul(tmp[:, :], wlup[:, :], SC[:, :])
        nc.vector.tensor_sub(densub[:, :], W[:, :], tmp[:, :])
        den_l = const.tile([128, NB], F32, tag="den_l")
        num_l = const.tile([128, NB], F32, tag="num_l")
        nc.scalar.activation(den_l[:, :], densub[:, :], AF.Ln)
        nc.scalar.activation(num_l[:, :], numin[:, :], AF.Ln)
        # loss rows: (den_l - num_l) / (cloc - 1 + KP)
        ctil = const.tile([128, NB], F32, tag="ctil")
        dinv = const.tile([128, NB], F32, tag="dinv")
        nc.vector.tensor_scalar_add(ctil[:, :], cloc[:, :], float(KP - 1))
        nc.vector.reciprocal(dinv[:, :], ctil[:, :])
        diff = const.tile([128, NB], F32, tag="diff")
        lossrow = const.tile([128, NB], F32, tag="lossrow")
        nc.vector.tensor_sub(diff[:, :], den_l[:, :], num_l[:, :])
        nc.vector.tensor_mul(lossrow[:, :], diff[:, :], dinv[:, :])

        # ---- reduce to a single partial ----------------------------------
        lsum = const.tile([128, 1], F32, tag="lsum")
        nc.vector.tensor_reduce(lsum[:, :], lossrow[:, :],
                                mybir.AxisListType.X, AL.add)
        psf = psum_pool.tile([128, QW], F32, tag="ps")
        nc.tensor.matmul(psf[0:1, 0:1], lsum[:, :],
                         ones_col[:, :], start=True, stop=True)
        outsb = const.tile([1, 1], F32, tag="outsb")
        nc.scalar.copy(outsb[0:1, 0:1], psf[0:1, 0:1])
        nc.sync.dma_start(out_d[:, :], outsb[0:1, 0:1])

        if debug_out:
            nc.sync.dma_start(dwbc_d[:, :], wbc[:, :])
            dfin = const.tile([128, 8 * NB], F32, tag="dfin")
            for i, t in enumerate([SC, W, eh, kpos, cloc, densub, numin,
                                   lossrow]):
                nc.vector.tensor_copy(dfin[:, i * NB:(i + 1) * NB], t[:, :])
            nc.sync.dma_start(dfin_d[:, :], dfin[:, :])

    nc.compile()
    return nc


# ---------------------------------------------------------------------------
# host-side marshalling
# ---------------------------------------------------------------------------

def make_inputs(q, k, y, cfg: Cfg):
    """Build the per-core input maps (pure layout/replication marshalling)."""
    N, D, KP = cfg.N, cfg.D, cfg.KP
    NL, NB, KC = cfg.NL, cfg.NB, cfg.KC
    q = np.asarray(q, dtype=np.float32)
    k = np.asarray(k, dtype=np.float32)
    y = np.asarray(y)

    qbf = q.astype(ml_dtypes.bfloat16)
    qTf = np.ascontiguousarray(qbf.T)           # [D, N]
    eyem = np.eye(128, dtype=np.float16)

    in_maps = []
    for r in range(NCORES):
        rows = slice(r * NL, (r + 1) * NL)
        roll = (np.arange(N) + r * NL) % N
        qT = np.ascontiguousarray(qTf[:, roll]).reshape(KC, 128, N)
        ybc = np.broadcast_to(y[roll].astype(np.float16)[None, :], (128, N)).copy()
        yrow = np.ascontiguousarray(
            y[rows].astype(np.float32).reshape(NB, 128).T)
        kr = np.ascontiguousarray(
            k[rows].reshape(NB, 128, KP * D)).astype(ml_dtypes.bfloat16)
        qr = np.ascontiguousarray(qbf[rows].reshape(NB, 128, D))
        mask8 = np.zeros((128, 1), dtype=np.float32)
        for p in range(64):
            c, s = divmod(p, NCORES)
            if (c + s) % NCORES == r:
                mask8[p, 0] = 1.0
        in_maps.append({
            "qT": qT, "kr": kr, "qr": qr, "ybc": ybc, "yrow": yrow,
            "eyem": eyem, "mask8": mask8,
        })
    return in_maps


_CACHE = {}


def _get_nc(cfg_key):
    if cfg_key not in _CACHE:
        cfg = Cfg()
        _CACHE[cfg_key] = (cfg, build_bass(cfg))
    return _CACHE[cfg_key]


def kernel(q, k, y, trace=False):
    cfg, nc = _get_nc("full")
    in_maps = make_inputs(q, k, y, cfg)
    res = run_bass_kernel_spmd(nc, in_maps, core_ids=list(range(NCORES)),
                               trace=trace)
    total = np.sum([res.results[r]["out"][0, 0] for r in range(NCORES)],
                   dtype=np.float64)
    out = np.asarray(total / cfg.N, dtype=np.float32)
    if trace:
        kernel.last_results = res
    return out


# revision 24
# speedup vs baseline: 1.8568x; 1.1194x over previous
"""Trainium2 Bass kernel for the supervised-contrastive loss (nn_KCL_69784628626020).

Strategy (8 NeuronCores, SPMD), v2:
  - Shard anchors (rows of q, k, y) across cores: 1024 rows/core.
  - Each core computes its [1024, 8192] slab of E = exp(q_loc @ q_full^T / TAU)
    on the tensor engine (bf16 operands, fp32 PSUM), in 4 column-"quarters"
    of 2048 per 128-row block.  The full q^T is SBUF-resident; the stationary
    (lhsT) operand is a slice of the same resident tensor, so DMA is ~8MB.
  - COLUMN ROTATION: core r's column order is rolled by r*1024 so that the
    self-similarity (diagonal) entry of local row-block b always lands in the
    static window [b*128, (b+1)*128).  A tiny eye-masked reduce extracts the
    exact stored E_ii per row; no per-tile diagonal masking is needed.
  - Per row i (sums include the diagonal; it cancels exactly):
        SC_i = sum_{y_j==y_i} E_ij        (DVE fused masked reduce / quarter)
        W_i  = sum_j w_j E_ij             (GpSimd product tile + ACT accum;
                                           w_j = 1/count(y_j) in f16)
        den_i = log(W_i - w_i*SC_i)       (diagonal + same-class terms cancel)
        num_i = log(kpos_i + SC_i - E_ii)
        loss_i = (den_i - num_i) / (count_i - 1 + K)
  - Class counts are computed on device (DVE label-equality reduces), shared
    via a tiny f16 AllGather of per-row 1/count, and re-broadcast into each
    core's rotated column order with one indirect DMA whose chunk indices are
    host-provided data (SPMD-safe).
  - kpos_i = sum_k exp(q_i . k_ik / TAU): GpSimd multiplies, ACT accumulates
    + exponentiates.
  - Final mean: ones-matmul partition reduction; host adds the 8 partials.
"""

import numpy as np
from contextlib import ExitStack

import concourse.bass as bass
import concourse.bacc as bacc
import concourse.tile as tile
from concourse import mybir
from concourse.bass import IndirectOffsetOnAxis
from concourse.bass_utils import run_bass_kernel_spmd
import ml_dtypes

F32 = mybir.dt.float32
F16 = mybir.dt.float16
BF16 = mybir.dt.bfloat16
I32 = mybir.dt.int32
AL = mybir.AluOpType
AF = mybir.ActivationFunctionType

TAU = 0.07
NCORES = 8


class Cfg:
    def __init__(self, N=8192, D=512, KP=8, NQ=4):
        self.N, self.D, self.KP, self.NQ = N, D, KP, NQ
        self.NL = N // NCORES      # rows per core
        self.NB = self.NL // 128   # 128-row blocks per core
        self.KC = D // 128         # contraction chunks
        self.QW = N // NQ          # column quarter width
        self.NCH = max(1, self.QW // 512)
        self.CW = self.QW // self.NCH   # matmul chunk width
        assert self.NL % 128 == 0 and self.QW % self.NCH == 0
        assert self.CW <= 512


def build_bass(cfg: Cfg, debug_out=False):
    N, D, KP, NQ = cfg.N, cfg.D, cfg.KP, cfg.NQ
    NL, NB, KC, QW, NCH, CW = cfg.NL, cfg.NB, cfg.KC, cfg.QW, cfg.NCH, cfg.CW
    NQT = NB * NQ              # total quarters
    WLAG = min(NQT - 1, 12)    # ACT-queue lag before W reductions start

    nc = bacc.Bacc("TRN2", target_bir_lowering=False, debug=False,
                   num_devices=NCORES)

    # ---- kernel I/O -------------------------------------------------------
    qT_d = nc.dram_tensor("qT", [KC, 128, N], BF16, kind="ExternalInput")
    kr_d = nc.dram_tensor("kr", [NB, 128, KP * D], BF16, kind="ExternalInput")
    qr_d = nc.dram_tensor("qr", [NB, 128, D], BF16, kind="ExternalInput")
    ybc_d = nc.dram_tensor("ybc", [128, N], F16, kind="ExternalInput")
    yrow_d = nc.dram_tensor("yrow", [128, NB], F32, kind="ExternalInput")
    eyem_d = nc.dram_tensor("eyem", [128, 128], F16, kind="ExternalInput")
    mask8_d = nc.dram_tensor("mask8", [128, 1], F32, kind="ExternalInput")
    out_d = nc.dram_tensor("out", [1, 1], F32, kind="ExternalOutput")
    if debug_out:
        dwbc_d = nc.dram_tensor("dwbc", [128, N], F16, kind="ExternalOutput")
        dfin_d = nc.dram_tensor("dfin", [128, 8 * NB], F32, kind="ExternalOutput")

    with tile.TileContext(nc) as tc, ExitStack() as ctx:
        const = ctx.enter_context(tc.tile_pool(name="const", bufs=1))
        ew_pool = ctx.enter_context(tc.tile_pool(name="ew", bufs=3))
        psum_pool = ctx.enter_context(tc.tile_pool(name="ps", bufs=2, space="PSUM"))
        k_pool = ctx.enter_context(tc.tile_pool(name="kp", bufs=1))
        q_pool = ctx.enter_context(tc.tile_pool(name="qp", bufs=2))
        ks_pool = ctx.enter_context(tc.tile_pool(name="ks", bufs=1))
        ws_pool = ctx.enter_context(tc.tile_pool(name="ws", bufs=2))
        dram = ctx.enter_context(tc.tile_pool(name="dram", bufs=1, space="DRAM"))

        # ---- resident constants ------------------------------------------
        # column-quarter-major loads so the PE can start after ~1/4 of qT
        qts = [const.tile([128, N], BF16, tag=f"qts{c}", name=f"qts{c}")
               for c in range(KC)]
        ybc = const.tile([128, N], F16, tag="ybc")
        yrow = const.tile([128, NB], F32, tag="yrow")
        nc.sync.dma_start(yrow[:, :], yrow_d[:, :])
        eyem = const.tile([128, 128], F16, tag="eyem")
        nc.sync.dma_start(eyem[:, :], eyem_d[:, :])
        mask8 = const.tile([128, 1], F32, tag="mask8")
        nc.sync.dma_start(mask8[:, :], mask8_d[:, :])
        nc.sync.dma_start(ybc[:, 0:QW], ybc_d[:, 0:QW])
        for c in range(KC):
            nc.sync.dma_start(qts[c][:, 0:QW], qT_d[c, :, 0:QW])
        nc.sync.dma_start(ybc[:, QW:N], ybc_d[:, QW:N])
        # k-path inputs early so GpSimd products (and ACT reduces) start soon
        kts = []
        qts_k = []
        for b in range(NB):
            kt = k_pool.tile([128, KP * D], BF16, tag="kt", name=f"kt{b}")
            nc.sync.dma_start(kt[:, :], kr_d[b, :, :])
            qt = q_pool.tile([128, D], BF16, tag="qt", name=f"qt{b}")
            nc.sync.dma_start(qt[:, :], qr_d[b, :, :])
            kts.append(kt)
            qts_k.append(qt)
        for qq in range(1, NQ):
            for c in range(KC):
                nc.sync.dma_start(qts[c][:, qq * QW:(qq + 1) * QW],
                                  qT_d[c, :, qq * QW:(qq + 1) * QW])
        ones_col = const.tile([128, 1], F32, tag="ones_col")
        nc.vector.memset(ones_col[:, :], 1.0)

        # accumulator slots
        scs = const.tile([128, NQT], F32, tag="scs")
        ws = const.tile([128, NQT], F32, tag="ws")
        eh = const.tile([128, NB], F32, tag="eh")
        kss = const.tile([128, NB * KP], F32, tag="kss")
        kpos = const.tile([128, NB], F32, tag="kpos")
        cloc = const.tile([128, NB], F32, tag="cloc")
        clocB = const.tile([128, NB], F32, tag="clocB")

        # ---- k-path products on GpSimd (kt, qt ready early) --------------
        kscrs = []
        for b in range(NB):
            kscr = ks_pool.tile([128, KP * D], BF16, tag="kscr", name=f"kscr{b}")
            for kk in range(KP):
                nc.gpsimd.tensor_tensor(
                    kscr[:, kk * D:(kk + 1) * D],
                    kts[b][:, kk * D:(kk + 1) * D], qts_k[b][:, :], op=AL.mult)
            kscrs.append(kscr)

        # ---- class counts on DVE (two halves per block) ------------------
        cnt_scr = const.tile([128, N // 2], F16, tag="cnt_scr")
        H = N // 2
        for b in range(NB):
            nc.vector.tensor_scalar(
                cnt_scr[:, :], ybc[:, 0:H], yrow[:, b:b + 1], None,
                op0=AL.is_equal, op1=AL.add, accum_out=cloc[:, b:b + 1])
        for b in range(NB):
            nc.vector.tensor_scalar(
                cnt_scr[:, :], ybc[:, H:N], yrow[:, b:b + 1], None,
                op0=AL.is_equal, op1=AL.add, accum_out=clocB[:, b:b + 1])
        nc.vector.tensor_add(cloc[:, :], cloc[:, :], clocB[:, :])

        # ---- w vector: wloc = 1/cloc; share via AllGather ----------------
        wloc = const.tile([128, NB], F32, tag="wloc")
        wloc16 = const.tile([128, NB], F16, tag="wloc16")
        wlup = const.tile([128, NB], F32, tag="wlup")
        nc.vector.reciprocal(wloc[:, :], cloc[:, :])
        nc.vector.tensor_copy(wloc16[:, :], wloc[:, :])
        # re-upcast of the f16 w for exact den cancellation
        nc.vector.tensor_copy(wlup[:, :], wloc16[:, :])

        # transpose wloc16 [128, NB] -> 32x32-block layout for a clean DMA
        wpad = const.tile([128, 32], F16, tag="wpad")
        nc.vector.memset(wpad[:, :], 0.0)
        nc.vector.tensor_copy(wpad[:, 0:NB], wloc16[:, :])
        wT = const.tile([128, 32], F16, tag="wT")
        nc.vector.transpose(wT[:, :], wpad[:, :])
        # wpart[0, i*128 + a*32 + j] = w(local row i*128+a*32+j) = wT[a*32+i, j]
        wpart = dram.tile([1, NL], F16)
        dst3 = wpart[:, :].rearrange("o (i a j) -> a i (o j)", i=NB, a=4, j=32)
        for a in range(4):
            nc.gpsimd.dma_start(dst3[a], wT[a * 32:a * 32 + NB, 0:32])
        # rotated share via ReduceScatter: core r contributes w_r to shard c
        # slot (r-c)%8; the scatter hands core c exactly its rotated vector.
        bufc = const.tile([128, NL], F16, tag="bufc")
        nc.gpsimd.dma_start(bufc[0:64, :],
                            wpart[0:1, :].partition_broadcast(64))
        nc.vector.tensor_scalar_mul(bufc[0:64, :], bufc[0:64, :],
                                    mask8[0:64, 0:1])
        bufc_d = dram.tile([1, 64 * NL], F16)
        nc.gpsimd.dma_start(
            bufc_d[:, :].rearrange("o (p g) -> p (o g)", p=64, g=NL),
            bufc[0:64, :])
        wrs_d = dram.tile([1, NCORES * NL], F16)
        nc.gpsimd.collective_compute(
            "ReduceScatter", AL.add,
            ins=[bufc_d[:, :].opt()],
            outs=[wrs_d[:, :].opt()],
            replica_groups=[list(range(NCORES))],
        )
        wbc = const.tile([128, N], F16, tag="wbc")
        nc.gpsimd.dma_start(wbc[:, :], wrs_d[0:1, :].partition_broadcast(128))

        # ---- main loop ----------------------------------------------------
        # PE: quarters of the score slab.  ACT: exp (+ k-path reduces early,
        # W reduces lagged).  DVE: SC masked reduce + diag extract.
        # GpSimd: W product tiles.
        sc_scr = const.tile([128, QW], F32, tag="sc_scr")
        eh_scr = const.tile([128, 128], BF16, tag="eh_scr")
        kdump = const.tile([128, D], BF16, tag="kdump")
        wdump = const.tile([128, QW], F32, tag="wdump")
        ew_tiles = {}
        wscr_tiles = {}

        kred_jobs = [(b, kk) for b in range(NB) for kk in range(KP)]
        kred_pos = 0
        kred_per_q = (len(kred_jobs) + WLAG - 2) // max(1, WLAG - 1)

        def emit_kred(n):
            nonlocal kred_pos
            for _ in range(n):
                if kred_pos >= len(kred_jobs):
                    return
                b, kk = kred_jobs[kred_pos]
                nc.scalar.activation(
                    kdump[:, :],
                    kscrs[b][:, kk * D:(kk + 1) * D],
                    AF.Copy, accum_out=kss[:, b * KP + kk: b * KP + kk + 1])
                kred_pos += 1

        def emit_wred(m):
            wscr = wscr_tiles.pop(m)
            nc.scalar.activation(
                wdump[:, :], wscr[:, :], AF.Copy,
                accum_out=ws[:, m:m + 1])

        for b in range(NB):
            ew = ew_pool.tile([128, N], BF16, tag="ew", name=f"ew{b}")
            ew_tiles[b] = ew
            for qq in range(NQ):
                m = b * NQ + qq
                ps = psum_pool.tile([128, QW], F32, tag="ps")
                for c in range(KC):
                    for ch in range(NCH):
                        o = ps[:, ch * CW:(ch + 1) * CW]
                        nc.tensor.matmul(
                            o,
                            qts[c][:, b * 128:(b + 1) * 128],
                            qts[c][:, qq * QW + ch * CW: qq * QW + (ch + 1) * CW],
                            start=(c == 0), stop=(c == KC - 1))
                ewq = ew[:, qq * QW:(qq + 1) * QW]
                nc.scalar.activation(ewq, ps[:, :], AF.Exp,
                                     scale=float(1.0 / TAU))
                # ACT queue fillers: k-path reduces early, W reduces lagged
                if m >= 1:
                    emit_kred(kred_per_q)
                if m >= WLAG:
                    emit_wred(m - WLAG)
                # SC: same-class row-sum (incl diag) on DVE
                nc.vector.scalar_tensor_tensor(
                    sc_scr[:, :], ybc[:, qq * QW:(qq + 1) * QW],
                    yrow[:, b:b + 1], ewq,
                    op0=AL.is_equal, op1=AL.mult,
                    accum_out=scs[:, m:m + 1])
                # W product tile on GpSimd
                wscr = ws_pool.tile([128, QW], F32, tag="wscr", name=f"wscr{m}")
                nc.gpsimd.tensor_tensor(
                    wscr[:, :], ewq, wbc[:, qq * QW:(qq + 1) * QW], op=AL.mult)
                wscr_tiles[m] = wscr
            # exact diagonal extraction from the static rotated window
            nc.vector.scalar_tensor_tensor(
                eh_scr[:, :], eyem[:, :], 1.0, ew[:, b * 128:(b + 1) * 128],
                op0=AL.mult, op1=AL.mult,
                accum_out=eh[:, b:b + 1])

        emit_kred(len(kred_jobs))
        for m in range(max(0, NQT - WLAG), NQT):
            emit_wred(m)

        # ---- k-path exps --------------------------------------------------
        ksse = const.tile([128, NB * KP], F32, tag="ksse")
        for b in range(NB):
            nc.scalar.activation(
                ksse[:, b * KP:(b + 1) * KP],
                kss[:, b * KP:(b + 1) * KP],
                AF.Exp, scale=float(1.0 / TAU),
                accum_out=kpos[:, b:b + 1])

        # ---- finalize (wide [128, NB] ops) --------------------------------
        SC = const.tile([128, NB], F32, tag="SC")
        W = const.tile([128, NB], F32, tag="W")
        nc.vector.tensor_reduce(
            SC[:, :], scs[:, :].rearrange("p (b q) -> p b q", b=NB, q=NQ),
            mybir.AxisListType.X, AL.add)
        nc.vector.tensor_reduce(
            W[:, :], ws[:, :].rearrange("p (b q) -> p b q", b=NB, q=NQ),
            mybir.AxisListType.X, AL.add)
        numin = const.tile([128, NB], F32, tag="numin")
        tmp = const.tile([128, NB], F32, tag="tmp")
        densub = const.tile([128, NB], F32, tag="densub")
        # num_in = kpos + (SC - eh)
        nc.vector.tensor_sub(tmp[:, :], SC[:, :], eh[:, :])
        nc.vector.tensor_add(numin[:, :], tmp[:, :], kpos[:, :])
        # den_in = W - w_i * SC  (w_i upcast from the same f16 used in wbc)
        nc.vector.tensor_mul(tmp[:, :], wlup[:, :], SC[:, :])
        nc.vector.tensor_sub(densub[:, :], W[:, :], tmp[:, :])
        den_l = const.tile([128, NB], F32, tag="den_l")
        num_l = const.tile([128, NB], F32, tag="num_l")
        nc.scalar.activation(den_l[:, :], densub[:, :], AF.Ln)
        nc.scalar.activation(num_l[:, :], numin[:, :], AF.Ln)
        # loss rows: (den_l - num_l) / (cloc - 1 + KP)
        ctil = const.tile([128, NB], F32, tag="ctil")
        dinv = const.tile([128, NB], F32, tag="dinv")
        nc.vector.tensor_scalar_add(ctil[:, :], cloc[:, :], float(KP - 1))
        nc.vector.reciprocal(dinv[:, :], ctil[:, :])
        diff = const.tile([128, NB], F32, tag="diff")
        lossrow = const.tile([128, NB], F32, tag="lossrow")
        nc.vector.tensor_sub(diff[:, :], den_l[:, :], num_l[:, :])
        nc.vector.tensor_mul(lossrow[:, :], diff[:, :], dinv[:, :])

        # ---- reduce to a single partial ----------------------------------
        lsum = const.tile([128, 1], F32, tag="lsum")
        nc.vector.tensor_reduce(lsum[:, :], lossrow[:, :],
                                mybir.AxisListType.X, AL.add)
        psf = psum_pool.tile([128, QW], F32, tag="ps")
        nc.tensor.matmul(psf[0:1, 0:1], lsum[:, :],
                         ones_col[:, :], start=True, stop=True)
        outsb = const.tile([1, 1], F32, tag="outsb")
        nc.scalar.copy(outsb[0:1, 0:1], psf[0:1, 0:1])
        nc.sync.dma_start(out_d[:, :], outsb[0:1, 0:1])

        if debug_out:
            nc.sync.dma_start(dwbc_d[:, :], wbc[:, :])
            dfin = const.tile([128, 8 * NB], F32, tag="dfin")
            for i, t in enumerate([SC, W, eh, kpos, cloc, densub, numin,
                                   lossrow]):
                nc.vector.tensor_copy(dfin[:, i * NB:(i + 1) * NB], t[:, :])
            nc.sync.dma_start(dfin_d[:, :], dfin[:, :])

    nc.compile()
    return nc


# ---------------------------------------------------------------------------
# host-side marshalling
# ---------------------------------------------------------------------------

def make_inputs(q, k, y, cfg: Cfg):
    """Build the per-core input maps (pure layout/replication marshalling)."""
    N, D, KP = cfg.N, cfg.D, cfg.KP
    NL, NB, KC = cfg.NL, cfg.NB, cfg.KC
    q = np.asarray(q, dtype=np.float32)
    k = np.asarray(k, dtype=np.float32)
    y = np.asarray(y)

    qbf = q.astype(ml_dtypes.bfloat16)
    qTf = np.ascontiguousarray(qbf.T)           # [D, N]
    eyem = np.eye(128, dtype=np.float16)

    in_maps = []
    for r in range(NCORES):
        rows = slice(r * NL, (r + 1) * NL)
        roll = (np.arange(N) + r * NL) % N
        qT = np.ascontiguousarray(qTf[:, roll]).reshape(KC, 128, N)
        ybc = np.broadcast_to(y[roll].astype(np.float16)[None, :], (128, N)).copy()
        yrow = np.ascontiguousarray(
            y[rows].astype(np.float32).reshape(NB, 128).T)
        kr = np.ascontiguousarray(
            k[rows].reshape(NB, 128, KP * D)).astype(ml_dtypes.bfloat16)
        qr = np.ascontiguousarray(qbf[rows].reshape(NB, 128, D))
        mask8 = np.zeros((128, 1), dtype=np.float32)
        for p in range(64):
            c, s = divmod(p, NCORES)
            if (c + s) % NCORES == r:
                mask8[p, 0] = 1.0
        in_maps.append({
            "qT": qT, "kr": kr, "qr": qr, "ybc": ybc, "yrow": yrow,
            "eyem": eyem, "mask8": mask8,
        })
    return in_maps


_CACHE = {}


def _get_nc(cfg_key):
    if cfg_key not in _CACHE:
        cfg = Cfg()
        _CACHE[cfg_key] = (cfg, build_bass(cfg))
    return _CACHE[cfg_key]


def kernel(q, k, y, trace=False):
    cfg, nc = _get_nc("full")
    in_maps = make_inputs(q, k, y, cfg)
    res = run_bass_kernel_spmd(nc, in_maps, core_ids=list(range(NCORES)),
                               trace=trace)
    total = np.sum([res.results[r]["out"][0, 0] for r in range(NCORES)],
                   dtype=np.float64)
    out = np.asarray(total / cfg.N, dtype=np.float32)
    if trace:
        kernel.last_results = res
    return out
